# revision 2
# baseline (speedup 1.0000x reference)
"""Bidirectional Mamba layer on 8 Trainium2 NeuronCores — v3.

Sharding: data-parallel over batch (8 batches -> 8 cores). Each core runs
both directions (fwd on x, bwd on time-reversed x) for its batch.

Engine plan (baseline v1 ~1021us):
  - GEMM1-u folds the causal depthwise conv: host precomputes
    Wk = diag(conv_w[:,k]) @ in_w_u, kernel sums 4 shifted GEMMs in PSUM,
    SiLU+bias straight out of PSUM -> ucT. No u tiles, no conv phase.
  - softplus = ln(1+exp(x+dtb)): 2 ACT ops.
  - scan, per direction, two 8-state halves (half-outer, chunk-mid,
    state-inner):
      per state: B/C broadcast via PE selector matmul -> PSUM, ACT copy ->
        SBUF bf16 (amortized over all 8 chunks)
      b = w*B: Pool TT (SBUF bf16); h: DVE scan; ytil = h*C: Pool/DVE TT
      y: PE identity matmuls accumulate ytil + diag(D)@uc in PSUM;
        half 0 flushes y to SBUF bf16, half 1 re-seeds from it.
      gate: DVE TT (y_psum f32 * silu(z)) -> g bf16 -> DRAM (round-trip
        to free SBUF for B/C residency)
  - G4: reload g, PE matmuls, copy, DMA out.
"""

import sys

sys.path.insert(0, "/opt/trn_rl_repo")

import numpy as np
import ml_dtypes

import concourse.bass as bass
import concourse.mybir as mybir
import bass_rust
from concourse import tile
from concourse.bass_utils import run_bass_kernel_spmd

BF16 = mybir.dt.bfloat16
F32 = mybir.dt.float32
AF = mybir.ActivationFunctionType
OP = mybir.AluOpType

D_MODEL = 512
D_INNER = 1024
D_STATE = 16
D_CONV = 4
DT_RANK = 32
BATCH = 8
SEQ = 1024

P = 128
NC_D = D_INNER // P  # 8 d-chunks
NN = SEQ // 512      # 2 psum-free chunks
SH = D_STATE // 2    # states per half-sweep

# of the scan muls, DVE takes DVE_MUL_NUM of every DVE_MUL_DEN (rest Pool)
DVE_MUL_NUM = 1
DVE_MUL_DEN = 9


def _dir_params(nc, d):
    return {
        "inwzT": nc.declare_dram_parameter(f"inwzT_{d}", [D_MODEL, D_INNER], BF16, isOutput=False),
        "wkm": nc.declare_dram_parameter(f"wkm_{d}", [P, NC_D * 16 * P], BF16, isOutput=False),
        "xpwT": nc.declare_dram_parameter(f"xpwT_{d}", [D_INNER, DT_RANK + 2 * D_STATE], BF16, isOutput=False),
        "dtwT": nc.declare_dram_parameter(f"dtwT_{d}", [DT_RANK, D_INNER], BF16, isOutput=False),
        "outwT": nc.declare_dram_parameter(f"outwT_{d}", [D_INNER, D_MODEL], BF16, isOutput=False),
        "A": nc.declare_dram_parameter(f"A_{d}", [D_INNER, D_STATE], F32, isOutput=False),
        "convb": nc.declare_dram_parameter(f"convb_{d}", [D_INNER, 1], F32, isOutput=False),
        "dtb": nc.declare_dram_parameter(f"dtb_{d}", [D_INNER, 1], F32, isOutput=False),
        "Dpd": nc.declare_dram_parameter(f"Dpd_{d}", [D_INNER, P], BF16, isOutput=False),
        "xT": nc.declare_dram_parameter(f"xT_{d}", [D_MODEL, SEQ], BF16, isOutput=False),
        "out": nc.declare_dram_parameter(f"out_{d}", [SEQ, D_MODEL], F32, isOutput=True),
    }


class Dir:
    pass


def build_bass():
    import contextlib

    nc = bass.Bass()
    params = {d: _dir_params(nc, d) for d in ("f", "b")}
    sel_d = nc.declare_dram_parameter("sel", [2 * D_STATE, 2 * D_STATE * P], BF16, isOutput=False)
    ident_d = nc.declare_dram_parameter("ident", [P, P], BF16, isOutput=False)

    mul_idx = [0]

    def flex_mul(out_ap, in0_ap, in1_ap):
        i = mul_idx[0]
        mul_idx[0] += 1
        if (i % DVE_MUL_DEN) < DVE_MUL_NUM:
            nc.vector.tensor_mul(out_ap, in0_ap, in1_ap)
        else:
            nc.gpsimd.tensor_mul(out_ap, in0_ap, in1_ap)

    with tile.TileContext(nc) as tc:
        with contextlib.ExitStack() as stack:
            cst = stack.enter_context(tc.tile_pool(name="cst", bufs=1))

            sel = cst.tile([2 * D_STATE, 2 * D_STATE * P], BF16, tag="sel", name="sel")
            nc.sync.dma_start(sel[:], sel_d[:])
            ident = cst.tile([P, P], BF16, tag="ident", name="ident")
            nc.sync.dma_start(ident[:], ident_d[:])

            dirs = {}
            for d in ("f", "b"):
                dd = Dir()
                dirs[d] = dd
                dd.ucT = [cst.tile([P, SEQ], BF16, tag=f"uc{d}{c}", name=f"uc{d}{c}") for c in range(NC_D)]
                dd.sz = [cst.tile([P, SEQ], BF16, tag=f"sz{d}{c}", name=f"sz{d}{c}") for c in range(NC_D)]
                dd.delta = [cst.tile([P, SEQ], BF16, tag=f"dl{d}{c}", name=f"dl{d}{c}") for c in range(NC_D)]
                dd.g = dd.delta  # gate output reuses delta storage (after last exp)
                dd.bc_bf = cst.tile([2 * D_STATE, SEQ], BF16, tag=f"bc{d}", name=f"bc{d}")

            # ============ phase generators ============
            def gemm_phase(d):
                """Yields after each unit of work; opens/closes its own pools."""
                dd = dirs[d]
                pp = params[d]
                with contextlib.ExitStack() as ph:
                    xp_pool = ph.enter_context(tc.tile_pool(name=f"xp{d}", bufs=1))
                    xT = [xp_pool.tile([P, SEQ + D_CONV - 1], BF16, tag=f"xT{d}{k}", name=f"xT{d}{k}") for k in range(4)]
                    for k in range(4):
                        nc.vector.memset(xT[k][:, 0:D_CONV - 1], 0.0)
                        nc.sync.dma_start(xT[k][:, D_CONV - 1:], pp["xT"][k * P:(k + 1) * P, :])
                    yield "x"

                    # ---- z half: silu(z) -> sz ----
                    with contextlib.ExitStack() as phz:
                        wz_pool = phz.enter_context(tc.tile_pool(name=f"wz{d}", bufs=1))
                        psz = phz.enter_context(tc.tile_pool(name=f"psz{d}", bufs=2, space="PSUM"))
                        wzT = [wz_pool.tile([P, D_INNER], BF16, tag=f"wz{d}{k}", name=f"wz{d}{k}") for k in range(4)]
                        for k in range(4):
                            nc.sync.dma_start(wzT[k][:], pp["inwzT"][k * P:(k + 1) * P, :])
                        for m in range(NC_D):
                            for n in range(NN):
                                pt = psz.tile([P, 512], F32, tag=f"g1z{d}", name=f"g1z{d}")
                                for k in range(4):
                                    nc.tensor.matmul(
                                        pt[:], wzT[k][:, m * P:(m + 1) * P],
                                        xT[k][:, D_CONV - 1 + n * 512: D_CONV - 1 + (n + 1) * 512],
                                        start=(k == 0), stop=(k == 3),
                                    )
                                nc.scalar.activation(dd.sz[m][:, n * 512:(n + 1) * 512], pt[:], AF.Silu)
                            yield "z"

                    # ---- u half with folded conv ----
                    with contextlib.ExitStack() as phu:
                        wu_pool = phu.enter_context(tc.tile_pool(name=f"wu{d}", bufs=2))
                        cb_pool = phu.enter_context(tc.tile_pool(name=f"cb{d}", bufs=1))
                        psu = phu.enter_context(tc.tile_pool(name=f"psu{d}", bufs=2, space="PSUM"))
                        convb = [cb_pool.tile([P, 1], F32, tag=f"cvb{d}{c}", name=f"cvb{d}{c}") for c in range(NC_D)]
                        for c in range(NC_D):
                            nc.sync.dma_start(convb[c][:], pp["convb"][c * P:(c + 1) * P, :])
                        for m in range(NC_D):
                            wkm = wu_pool.tile([P, 16 * P], BF16, tag=f"wkm{d}", name=f"wkm{d}")
                            nc.sync.dma_start(
                                wkm[:], pp["wkm"][:, m * 16 * P:(m + 1) * 16 * P]
                            )
                            for n in range(NN):
                                pt = psu.tile([P, 512], F32, tag=f"g1u{d}", name=f"g1u{d}")
                                first = True
                                for t in range(D_CONV):
                                    for k in range(4):
                                        j = t * 4 + k
                                        nc.tensor.matmul(
                                            pt[:], wkm[:, j * P:(j + 1) * P],
                                            xT[k][:, t + n * 512: t + n * 512 + 512],
                                            start=first, stop=(t == D_CONV - 1 and k == 3),
                                        )
                                        first = False
                                nc.scalar.activation(
                                    dd.ucT[m][:, n * 512:(n + 1) * 512], pt[:],
                                    AF.Silu, bias=convb[m][:, 0:1],
                                )
                            yield "u"

                    # ---- GEMM2 + GEMM3 + softplus ----
                    with contextlib.ExitStack() as ph2:
                        w2_pool = ph2.enter_context(tc.tile_pool(name=f"w2{d}", bufs=1))
                        ps2 = ph2.enter_context(tc.tile_pool(name=f"ps2{d}", bufs=1, space="PSUM"))
                        ps3 = ph2.enter_context(tc.tile_pool(name=f"ps3{d}", bufs=1, space="PSUM"))
                        sp_t = ph2.enter_context(tc.tile_pool(name=f"sp{d}", bufs=1))
                        xpwT = [w2_pool.tile([P, 64], BF16, tag=f"xpw{d}{c}", name=f"xpw{d}{c}") for c in range(NC_D)]
                        dtb = [w2_pool.tile([P, 1], F32, tag=f"dtb{d}{c}", name=f"dtb{d}{c}") for c in range(NC_D)]
                        for c in range(NC_D):
                            nc.sync.dma_start(xpwT[c][:], pp["xpwT"][c * P:(c + 1) * P, :])
                            nc.sync.dma_start(dtb[c][:], pp["dtb"][c * P:(c + 1) * P, :])
                        dtwT = w2_pool.tile([DT_RANK, D_INNER], BF16, tag=f"dtw{d}", name=f"dtw{d}")
                        nc.sync.dma_start(dtwT[:], pp["dtwT"][:])
                        dt_bf = w2_pool.tile([DT_RANK, SEQ], BF16, tag=f"dtv{d}", name=f"dtv{d}")

                        for n in range(NN):
                            pt = ps2.tile([64, 512], F32, tag=f"g2{d}", name=f"g2{d}")
                            for c in range(NC_D):
                                nc.tensor.matmul(
                                    pt[:], xpwT[c][:], dd.ucT[c][:, n * 512:(n + 1) * 512],
                                    start=(c == 0), stop=(c == NC_D - 1),
                                )
                            nc.vector.tensor_copy(dt_bf[:, n * 512:(n + 1) * 512], pt[0:DT_RANK, :])
                            nc.vector.tensor_copy(dd.bc_bf[:, n * 512:(n + 1) * 512], pt[DT_RANK:64, :])
                            yield "g2"

                        for m in range(NC_D):
                            pt = ps3.tile([P, SEQ], F32, tag=f"g3{d}", name=f"g3{d}")
                            for n in range(NN):
                                nc.tensor.matmul(
                                    pt[:, n * 512:(n + 1) * 512],
                                    dtwT[:, m * P:(m + 1) * P],
                                    dt_bf[:, n * 512:(n + 1) * 512],
                                    start=True, stop=True,
                                )
                            ex = sp_t.tile([P, SEQ], F32, tag=f"spe{d}", name=f"spe{d}")
                            nc.scalar.activation(ex[:], pt[:], AF.Exp, bias=dtb[m][:, 0:1])
                            nc.scalar.activation(dd.delta[m][:], ex[:], AF.Ln, bias=1.0)
                            yield "g3"

                yield "ready"

            def scan_phase(d):
                dd = dirs[d]
                pp = params[d]
                with contextlib.ExitStack() as ph:
                    sc_w = ph.enter_context(tc.tile_pool(name=f"scw{d}", bufs=1))
                    bcps = ph.enter_context(tc.tile_pool(name=f"bcps{d}", bufs=1, space="PSUM"))
                    yps = ph.enter_context(tc.tile_pool(name=f"yps{d}", bufs=1, space="PSUM"))
                    bcs = ph.enter_context(tc.tile_pool(name=f"bcs{d}", bufs=1))
                    ab = ph.enter_context(tc.tile_pool(name=f"ab{d}", bufs=3))
                    hp = ph.enter_context(tc.tile_pool(name=f"hp{d}", bufs=3))
                    wp = ph.enter_context(tc.tile_pool(name=f"wp{d}", bufs=2))
                    yh = ph.enter_context(tc.tile_pool(name=f"yh{d}", bufs=1))

                    A_sb = [sc_w.tile([P, D_STATE], F32, tag=f"A{d}{c}", name=f"A{d}{c}") for c in range(NC_D)]
                    Dpd = [sc_w.tile([P, P], BF16, tag=f"Dp{d}{c}", name=f"Dp{d}{c}") for c in range(NC_D)]
                    for c in range(NC_D):
                        nc.sync.dma_start(A_sb[c][:], pp["A"][c * P:(c + 1) * P, :])
                        nc.sync.dma_start(Dpd[c][:], pp["Dpd"][c * P:(c + 1) * P, :])
                    yield

                    yhalf = [None] * NC_D
                    for half in range(2):
                        ss = list(range(half * SH, (half + 1) * SH))
                        Bsb, Csb = {}, {}
                        for s in ss:
                            bp = bcps.tile([P, SEQ], F32, tag=f"bcp{d}", name=f"bcp{d}")
                            for n in range(NN):
                                nc.tensor.matmul(
                                    bp[:, n * 512:(n + 1) * 512], sel[:, s * P:(s + 1) * P],
                                    dd.bc_bf[:, n * 512:(n + 1) * 512],
                                    start=True, stop=True,
                                )
                            Bsb[s] = bcs.tile([P, SEQ], BF16, tag=f"Bs{d}{s % SH}", name=f"Bs{d}{s % SH}")
                            nc.scalar.copy(Bsb[s][:], bp[:])
                            cp_ = bcps.tile([P, SEQ], F32, tag=f"bcp{d}", name=f"bcp{d}")
                            for n in range(NN):
                                nc.tensor.matmul(
                                    cp_[:, n * 512:(n + 1) * 512],
                                    sel[:, (D_STATE + s) * P:(D_STATE + s + 1) * P],
                                    dd.bc_bf[:, n * 512:(n + 1) * 512],
                                    start=True, stop=True,
                                )
                            Csb[s] = bcs.tile([P, SEQ], BF16, tag=f"Cs{d}{s % SH}", name=f"Cs{d}{s % SH}")
                            nc.scalar.copy(Csb[s][:], cp_[:])
                        yield

                        for c in range(NC_D):
                            ypt = yps.tile([P, SEQ], F32, tag=f"y{d}", name=f"y{d}")
                            for n in range(NN):
                                sl = slice(n * 512, (n + 1) * 512)
                                if half == 0:
                                    nc.tensor.matmul(ypt[:, sl], Dpd[c][:], dd.ucT[c][:, sl], start=True, stop=False)
                                else:
                                    nc.tensor.matmul(ypt[:, sl], ident[:], yhalf[c][:, sl], start=True, stop=False)
                            wt = wp.tile([P, SEQ], BF16, tag=f"w{d}", name=f"w{d}")
                            nc.vector.tensor_mul(wt[:], dd.delta[c][:], dd.ucT[c][:])
                            for s in ss:
                                a_t = ab.tile([P, SEQ], BF16, tag=f"a{d}", name=f"a{d}")
                                nc.scalar.activation(
                                    a_t[:], dd.delta[c][:], AF.Exp, scale=A_sb[c][:, s:s + 1]
                                )
                                b_t = ab.tile([P, SEQ], BF16, tag=f"b{d}", name=f"b{d}")
                                flex_mul(b_t[:], wt[:], Bsb[s][:])
                                h_t = hp.tile([P, SEQ], BF16, tag=f"h{d}", name=f"h{d}")
                                nc.vector.tensor_tensor_scan(
                                    h_t[:], a_t[:], b_t[:], 0.0, op0=OP.mult, op1=OP.add
                                )
                                yt = hp.tile([P, SEQ], BF16, tag=f"yt{d}", name=f"yt{d}")
                                flex_mul(yt[:], h_t[:], Csb[s][:])
                                for n in range(NN):
                                    sl = slice(n * 512, (n + 1) * 512)
                                    nc.tensor.matmul(
                                        ypt[:, sl], ident[:], yt[:, sl],
                                        start=False, stop=(s == ss[-1]),
                                    )
                            if half == 0:
                                yhalf[c] = yh.tile([P, SEQ], BF16, tag=f"yh{d}{c}", name=f"yh{d}{c}")
                                nc.vector.tensor_copy(yhalf[c][:], ypt[:])
                            else:
                                nc.vector.tensor_mul(dd.g[c][:], ypt[:], dd.sz[c][:])
                            yield

            def g4_phase(d):
                dd = dirs[d]
                pp = params[d]
                with contextlib.ExitStack() as ph:
                    gw = ph.enter_context(tc.tile_pool(name=f"gw{d}", bufs=1))
                    ps4 = ph.enter_context(tc.tile_pool(name=f"ps4{d}", bufs=2, space="PSUM"))
                    o_pool = ph.enter_context(tc.tile_pool(name=f"o{d}", bufs=3))
                    outwT = [gw.tile([P, D_MODEL], BF16, tag=f"ow{d}{c}", name=f"ow{d}{c}") for c in range(NC_D)]
                    for c in range(NC_D):
                        nc.sync.dma_start(outwT[c][:], pp["outwT"][c * P:(c + 1) * P, :])
                    yield
                    for m in range(SEQ // P):
                        pt = ps4.tile([P, D_MODEL], F32, tag=f"g4{d}", name=f"g4{d}")
                        for c in range(NC_D):
                            nc.tensor.matmul(
                                pt[:], dd.g[c][:, m * P:(m + 1) * P], outwT[c][:],
                                start=(c == 0), stop=(c == NC_D - 1),
                            )
                        ot = o_pool.tile([P, D_MODEL], F32, tag=f"ot{d}", name=f"ot{d}")
                        nc.vector.tensor_copy(ot[:], pt[:])
                        nc.sync.dma_start(pp["out"][m * P:(m + 1) * P, :], ot[:])
                        yield

            def drain(gen):
                for _ in gen:
                    pass

            def interleave(primary, secondary, ratio):
                """Emit `ratio` units of secondary per unit of primary."""
                import itertools
                p_done = s_done = False
                pi = iter(primary)
                si = iter(secondary)
                while not (p_done and s_done):
                    if not p_done:
                        try:
                            next(pi)
                        except StopIteration:
                            p_done = True
                    if not s_done:
                        for _ in range(ratio):
                            try:
                                next(si)
                            except StopIteration:
                                s_done = True
                                break

            import itertools

            def until_ready(gen):
                for tag in gen:
                    if tag == "ready":
                        return

            # f: x/u/G23 (scan prerequisites), then f-scan interleaved with
            # f's deferred z + all of b's GEMMs; then b-scan with f's G4.
            def take_until_ready(gen):
                for tag in gen:
                    yield tag
                    if tag == "ready":
                        return

            gf = gemm_phase("f")
            until_ready(gf)
            gb = gemm_phase("b")
            interleave(scan_phase("f"), itertools.chain(gf, take_until_ready(gb)), 2)
            interleave(scan_phase("b"), itertools.chain(gb, g4_phase("f")), 1)
            drain(g4_phase("b"))

    _split_excess_waits(nc)
    return nc


def _split_excess_waits(nc):
    """walrus accepts at most one sync-wait per instruction (two for
    EventSemaphore); hoist the excess onto injected same-engine NoOps."""
    for f in nc.m.functions:
        for bb in f.blocks:
            new_insts = []
            for inst in bb.instructions:
                si = inst.sync_info
                cap = 2 if isinstance(inst, mybir.InstEventSemaphore) else 1
                if si is not None and len(si.on_wait) > cap:
                    waits = list(si.on_wait)
                    for i, w in enumerate(waits[:-cap]):
                        nop = mybir.InstNoOp(
                            name=f"{inst.name}-wsplit{i}", ins=[], outs=[]
                        )
                        nop.engine = inst.engine
                        nop.sync_info = bass_rust.SyncInfo(on_wait=[w], on_update=[])
                        new_insts.append(nop)
                    inst.sync_info = bass_rust.SyncInfo(
                        on_wait=waits[-cap:], on_update=list(si.on_update)
                    )
                new_insts.append(inst)
            try:
                bb.instructions = new_insts
            except Exception:
                bb.instructions.clear()
                bb.instructions.extend(new_insts)


def _prep_dir(w):
    bf = ml_dtypes.bfloat16
    in_w, conv_w, conv_b, xp_w, dt_w, dt_b, A_log, Dp, out_w = w
    in_wT = np.ascontiguousarray(np.asarray(in_w, np.float32).T)  # [512, 2048]
    inwuT = in_wT[:, 0:D_INNER]
    cw = np.asarray(conv_w, np.float32)
    # wkm: SBUF layout [x-row partition, per-m 16 (t,k) blocks of d-cols]
    wkm = np.empty((P, NC_D * 16 * P), np.float32)
    for m in range(NC_D):
        for t in range(D_CONV):
            for k in range(4):
                j = t * 4 + k
                blk = inwuT[k * P:(k + 1) * P, m * P:(m + 1) * P] * cw[m * P:(m + 1) * P, t][None, :]
                wkm[:, (m * 16 + j) * P:(m * 16 + j + 1) * P] = blk
    Dpd = np.zeros((D_INNER, P), np.float32)
    Dv = np.asarray(Dp, np.float32)
    for c in range(NC_D):
        np.fill_diagonal(Dpd[c * P:(c + 1) * P, :], Dv[c * P:(c + 1) * P])
    return {
        "inwzT": np.ascontiguousarray(in_wT[:, D_INNER:]).astype(bf),
        "wkm": np.ascontiguousarray(wkm).astype(bf),
        "xpwT": np.ascontiguousarray(np.asarray(xp_w).T).astype(bf),
        "dtwT": np.ascontiguousarray(np.asarray(dt_w).T).astype(bf),
        "outwT": np.ascontiguousarray(np.asarray(out_w).T).astype(bf),
        "A": np.ascontiguousarray(-np.exp(np.asarray(A_log, np.float64))).astype(np.float32),
        "convb": np.asarray(conv_b, np.float32).reshape(D_INNER, 1),
        "dtb": np.asarray(dt_b, np.float32).reshape(D_INNER, 1),
        "Dpd": Dpd.astype(bf),
    }


_CACHED = {}


def kernel(
    x,
    in_w_f, conv_w_f, conv_b_f, xp_w_f, dt_w_f, dt_b_f, A_log_f, D_f, out_w_f,
    in_w_b, conv_w_b, conv_b_b, xp_w_b, dt_w_b, dt_b_b, A_log_b, D_b, out_w_b,
):
    bf = ml_dtypes.bfloat16
    x = np.asarray(x, dtype=np.float32)

    if "nc" not in _CACHED:
        _CACHED["nc"] = build_bass()
    nc = _CACHED["nc"]

    wf = _prep_dir((in_w_f, conv_w_f, conv_b_f, xp_w_f, dt_w_f, dt_b_f,
                    A_log_f, D_f, out_w_f))
    wb = _prep_dir((in_w_b, conv_w_b, conv_b_b, xp_w_b, dt_w_b, dt_b_b,
                    A_log_b, D_b, out_w_b))
    sel = np.zeros((2 * D_STATE, 2 * D_STATE * P), np.float32)
    for i in range(2 * D_STATE):
        sel[i, i * P:(i + 1) * P] = 1.0
    sh = {
        "sel": sel.astype(bf),
        "ident": np.eye(P, dtype=np.float32).astype(bf),
    }

    in_maps = []
    for b in range(BATCH):
        m = dict(sh)
        for d, wd in (("f", wf), ("b", wb)):
            for k, v in wd.items():
                m[f"{k}_{d}"] = v
        m["xT_f"] = np.ascontiguousarray(x[b].T).astype(bf)
        m["xT_b"] = np.ascontiguousarray(x[b][::-1].T).astype(bf)
        in_maps.append(m)

    res = run_bass_kernel_spmd(nc, in_maps, core_ids=list(range(BATCH)))
    out = np.empty((BATCH, SEQ, D_MODEL), np.float32)
    for b in range(BATCH):
        rb = res.results[b]
        out[b] = rb["out_f"] + rb["out_b"][::-1]
    return out
